# revision 3
# baseline (speedup 1.0000x reference)
"""Trainium2 Bass kernel for nn_GN_89266600280080.

Computes, for output[B,O], input[B,D], weights[O]:
    dl_dW = (1/B) * (output * weights)^T @ input        # [O, D]
    gw    = sqrt(sum(dl_dW^2, axis=1))                  # [O]

Strategy (8 NeuronCores, data-parallel over batch):
  - shard output/input on dim 0 across the 8 cores (B_loc = B/8 = 4096)
  - per core: M_partial = output_loc^T @ input_loc accumulated in PSUM
    via 128-deep K tiles on the tensor engine (weights folding deferred)
  - AllReduce(add) the [O, D] partial across cores
  - per core: ss[o] = sum_d M[o,d]^2  (DVE fused square+reduce), then
    gw[o] = sqrt(ss[o] * (weights[o]/B)^2)  on the scalar engine
  - host takes core 0's gw
"""

import sys
import numpy as np

for _p in ("/opt/trn_rl_repo", "/root/.axon_site/_ro/trn_rl_repo"):
    if _p not in sys.path:
        sys.path.insert(0, _p)

B, O, D = 32768, 32, 1024
N_CORES = 8
B_LOC = B // N_CORES
P = 128                 # partitions per K tile
NMM = 512               # moving-operand free dim per fp32 matmul


def build(b_loc=B_LOC, n_cores=N_CORES, b_total=B, ch=4, n_iters=1):
    """Build + compile the per-core Bass program. Returns the Bacc object."""
    import concourse.bacc as bacc
    import concourse.tile as tile
    import concourse.mybir as mybir

    f32 = mybir.dt.float32
    kt = b_loc // P          # K tiles per core
    assert b_loc % P == 0 and kt % ch == 0
    nh = D // NMM

    nc = bacc.Bacc("TRN2", target_bir_lowering=False, debug=False,
                   num_devices=n_cores)

    out_d = nc.dram_tensor("output", [b_loc, O], f32, kind="ExternalInput")
    in_d = nc.dram_tensor("input", [b_loc, D], f32, kind="ExternalInput")
    w_d = nc.dram_tensor("weights", [O], f32, kind="ExternalInput")
    gw_d = nc.dram_tensor("gw", [O], f32, kind="ExternalOutput")

    out_ap = out_d.ap().rearrange("(n p) o -> p n o", p=P)
    in_ap = in_d.ap().rearrange("(n p) d -> p n d", p=P)

    with tile.TileContext(nc) as tc:
        with (
            tc.tile_pool(name="wout", bufs=2) as wout_pool,
            tc.tile_pool(name="rhs", bufs=3) as rhs_pool,
            tc.tile_pool(name="ps", bufs=2, space="PSUM") as psum_pool,
            tc.tile_pool(name="misc", bufs=2) as misc,
            tc.tile_pool(name="dram", bufs=2, space="DRAM") as dram_pool,
        ):
            for _it in range(n_iters):
                # stationary operand: all local w_out rows, [128, kt, O]
                wout = wout_pool.tile([P, kt, O], f32)
                nc.sync.dma_start(wout[:], out_ap)

                w_sb = misc.tile([O, 1], f32)
                nc.sync.dma_start(
                    w_sb[:], w_d.ap().rearrange("(p one) -> p one", one=1))
                # w2 = (w / B)^2 exactly on DVE
                w_n = misc.tile([O, 1], f32)
                nc.vector.tensor_scalar_mul(w_n[:], w_sb[:], 1.0 / b_total)
                w2 = misc.tile([O, 1], f32)
                nc.vector.tensor_mul(w2[:], w_n[:], w_n[:])

                psum = psum_pool.tile([O, D], f32)
                for c in range(kt // ch):
                    rhs = rhs_pool.tile([P, ch, D], f32)
                    nc.sync.dma_start(
                        rhs[:], in_ap[:, c * ch:(c + 1) * ch, :])
                    for j in range(ch):
                        k = c * ch + j
                        for h in range(nh):
                            nc.tensor.matmul(
                                psum[:, h * NMM:(h + 1) * NMM],
                                wout[:, k, :],
                                rhs[:, j, h * NMM:(h + 1) * NMM],
                                start=(k == 0),
                                stop=(k == kt - 1),
                            )

                # PSUM -> SBUF -> DRAM (DMA cannot read PSUM)
                part_sb = misc.tile([O, D], f32)
                nc.vector.tensor_copy(part_sb[:], psum[:])
                part_dram = dram_pool.tile([O, D], f32)
                nc.sync.dma_start(part_dram[:], part_sb[:])

                red_dram = dram_pool.tile([O, D], f32)
                nc.gpsimd.collective_compute(
                    "AllReduce",
                    mybir.AluOpType.add,
                    replica_groups=[list(range(n_cores))],
                    ins=[part_dram.opt()],
                    outs=[red_dram.opt()],
                )

                red_sb = misc.tile([O, D], f32)
                nc.sync.dma_start(red_sb[:], red_dram[:])

                # ss[o] = sum_d red[o,d]^2  (tensor_tensor_reduce crashes the
                # runtime here, so square + reduce separately on DVE)
                sq = misc.tile([O, D], f32)
                nc.vector.tensor_mul(sq[:], red_sb[:], red_sb[:])
                ss = misc.tile([O, 1], f32)
                nc.vector.reduce_sum(ss[:], sq[:], axis=mybir.AxisListType.X)

                # gw = sqrt(ss * (w/B)^2)
                gw_sb = misc.tile([O, 1], f32)
                nc.scalar.activation(
                    gw_sb[:], ss[:], mybir.ActivationFunctionType.Sqrt,
                    bias=0.0, scale=w2[:])
                nc.sync.dma_start(
                    gw_d.ap().rearrange("(p one) -> p one", one=1), gw_sb[:])

    nc.compile()
    return nc


_CACHE = {}


def _get_nc():
    if "nc" not in _CACHE:
        _CACHE["nc"] = build()
    return _CACHE["nc"]


def kernel(output, input, weights):
    from concourse.bass_utils import run_bass_kernel_spmd

    output = np.asarray(output, dtype=np.float32)
    input = np.asarray(input, dtype=np.float32)
    weights = np.asarray(weights, dtype=np.float32)

    nc = _get_nc()
    in_maps = [
        {
            "output": output[c * B_LOC:(c + 1) * B_LOC],
            "input": input[c * B_LOC:(c + 1) * B_LOC],
            "weights": weights,
        }
        for c in range(N_CORES)
    ]
    res = run_bass_kernel_spmd(nc, in_maps, list(range(N_CORES)))
    return np.asarray(res.results[0]["gw"], dtype=np.float32).reshape(O)
